# revision 42
# baseline (speedup 1.0000x reference)
"""DeepseekMoE block-quantized MoE kernel for 8 Trainium2 NeuronCores.

Strategy (expert-parallel with host-side dispatch):
  - The routing table (selected_experts) is known on the host before launch,
    so the all-to-all "dispatch" is done on the host: for each expert e we
    gather the unique tokens routed to it (dedup across the top-k slots),
    transpose to [H, n_e], and split across per-core job slots.
  - Each core hosts 2 experts.  The SPMD program runs S jobs of fixed
    widths W[0..S-1]; each job slot is fed one expert's weights and a
    window of its token columns.  With S=4 narrow slots a core can split
    its two experts across a per-core bipartition of the slots, so the
    total padded width per core approaches the max expert-pair sum
    (2656 vs 2704 for the classic 2-slot layout) - all host-side data
    routing, no program branching.
  - Each job runs a dense 3-matmul MLP (gate/up -> silu*up -> down) in
    x^T / act^T layout so no on-device transposes are needed.
  - Block-dequantization (w * repeat(s, 128)) is folded into the host-side
    weight preparation.
  - All tensors are bf16 on device (PSUM accumulation stays fp32): same
    1 col/cycle PE streaming rate as float32r, but half the DMA bytes and
    SBUF footprint, and LDWEIGHTS gets fast-weight-load (2x).
  - Weights are stored in DRAM in slab-major layout ([slab, 128, free])
    so each weight-slab DMA is 128 descriptors of 4KB/2.8KB contiguous
    instead of 2048 descriptors of 512B (the DMA engines are
    descriptor-rate-bound, not byte-bound).
  - x loads are one 3D-AP DMA per chunk (DMA issue costs ~600ns each on
    the issuing engine; packets of one DMA round-robin across all 16
    engines anyway).  y writes are batched per h-tile and issued from the
    Scalar engine's DGE queue so their CAST-waits never head-of-line
    block the input loads on the Sync queue.
  - ~100 warmup matmuls on scratch tiles run while the first inputs
    stream in, so the PE's HAM clock gate reaches 2.4 GHz before real
    work starts.
  - The host scatters the per-slot outputs back to [T, K, H].
"""

import math

import numpy as np

T = 4096
TOPK = 6
E = 16
H = 2048
I = 1408
BS = 128           # quant block size
HT = H // 128      # 16 h-tiles
IT = I // 128      # 11 i-tiles
NCORES = 8
# SBUF bound per partition: 62*W bytes of W-dependent tiles (x, acts, yc)
# plus ~60KB of weight staging must stay under ~200KB.
MAX_W = 2240

_BUILT = {}
LAST_RESULTS = None  # stashed BassKernelResults for external harnesses


def _chunk_plan(width):
    """Split `width` columns into PSUM-bank-sized chunks (<=512), each >=256
    when width allows (keeps every matmul well above the LDWEIGHTS shadow)."""
    if width <= 512:
        return [(0, width)]
    n = -(-width // 512)
    base = (width // n) // 8 * 8
    rem8 = (width - n * base) // 8
    out, off = [], 0
    for j in range(n):
        w = base + (8 if j < rem8 else 0)
        if j == n - 1:
            w = width - off
        out.append((off, w))
        off += w
    return out


def _build(jobs, CT):
    """Build the SPMD Bass program.  `jobs` is a tuple of
    (slot, col_offset, width): each job runs one expert slot's MLP over a
    window of `width` token columns; CT is the column capacity of xt/yt."""
    import concourse.bacc as bacc
    import concourse.mybir as mybir
    from concourse.bass import ts
    from concourse.tile import TileContext

    f32 = mybir.dt.float32
    bf16 = mybir.dt.bfloat16
    AF = mybir.ActivationFunctionType
    import os as _os

    act_fn = (
        AF.Sigmoid if _os.environ.get("KERNEL_SIM_SIGMOID") else AF.Silu
    )  # CoreSim lacks Silu; HW path always uses Silu

    S = max(j[0] for j in jobs) + 1
    nc = bacc.Bacc()
    xt = nc.declare_dram_parameter("xt", [S, HT, 128, CT], bf16, isOutput=False)
    # slab-major weights: w0t/w1t[s, it, p, hb*128+j] = Wdq[it*128+j, hb*128+p]
    w0t = nc.declare_dram_parameter("w0t", [S, IT, 128, H], bf16, isOutput=False)
    w1t = nc.declare_dram_parameter("w1t", [S, IT, 128, H], bf16, isOutput=False)
    # w2t[s, ht, p, it*128+j] = W2dq[ht*128+j, it*128+p]
    w2t = nc.declare_dram_parameter("w2t", [S, HT, 128, I], bf16, isOutput=False)
    yt = nc.declare_dram_parameter("yt", [S, HT, 128, CT], bf16, isOutput=True)

    with TileContext(nc) as tc:
        with (
            tc.tile_pool(name="xp", bufs=1) as xp,
            tc.tile_pool(name="ap", bufs=1) as apool,
            tc.tile_pool(name="wp", bufs=2) as wp,
            tc.tile_pool(name="yp", bufs=3) as yp,
            tc.tile_pool(name="ps", bufs=2, space="PSUM") as ps,
        ):
            # PE warmup: dummy matmuls on scratch tiles while the first x/w
            # DMAs stream in, so the HAM clock gate reaches 2.4 GHz before
            # real matmuls start (otherwise the first ~3.4us run at 1.2 GHz).
            warm_sb = xp.tile([128, 192], bf16, tag="warm")
            nc.vector.memset(warm_sb, 0.0)
            warm_ps = ps.tile([128, 512], f32, tag="o", bufs=4)
            def warm_burst(n):
                for _ in range(n):
                    nc.tensor.matmul(
                        warm_ps[:, :192], warm_sb[:, :128], warm_sb,
                        start=True, stop=True,
                    )

            warm_burst(28)

            # "gate" loads for job j: w0/w1 slab 0 + all x chunks.  Emitted
            # during job j-1's Phase B (after its w2 slabs) so the transfers
            # complete before job j's first matmul group needs them.
            gates = {}

            def emit_gate(j):
                s_, co_, W_ = jobs[j]
                ch = _chunk_plan(W_)
                if j == 0:
                    # job 0's x in four independent tiles (4 h-tiles each):
                    # Tile tracks dependencies per tile, so the first matmul
                    # group starts after w0 + one 315KB part instead of the
                    # whole chunk (real work from ~11us instead of ~18us)
                    parts = [
                        xp.tile([128, 4 * W_], bf16, tag=f"xg{pp}", name=f"xg{pp}_0")
                        for pp in range(4)
                    ]

                    def load_part_chunk(pp, ci):
                        c0, cw = ch[ci]
                        nc.sync.dma_start(
                            out=parts[pp].rearrange("p (h w) -> p h w", h=4)[
                                :, :, c0 : c0 + cw
                            ],
                            in_=xt[
                                s_, 4 * pp : 4 * pp + 4, :, co_ + c0 : co_ + c0 + cw
                            ].rearrange("h p w -> p h w"),
                        )

                    w0sj = wp.tile([128, H], bf16, tag="w0", name=None)
                    nc.sync.dma_start(out=w0sj, in_=w0t[s_, 0])
                    for pp in range(4):
                        load_part_chunk(pp, 0)
                    w1sj = wp.tile([128, H], bf16, tag="w1", name=None)
                    nc.sync.dma_start(out=w1sj, in_=w1t[s_, 0])
                    for ci in range(1, len(ch)):
                        for pp in range(4):
                            load_part_chunk(pp, ci)

                    def xslj(h, c0, cw, _p=parts, _W=W_):
                        return _p[h // 4][
                            :, (h % 4) * _W + c0 : (h % 4) * _W + c0 + cw
                        ]

                    gates[j] = (xslj, w0sj, w1sj)
                    return
                xsj = xp.tile([128, HT * W_], bf16, tag="x", name=f"x_{j}")

                def load_x_chunk(ci):
                    c0, cw = ch[ci]
                    nc.sync.dma_start(
                        out=xsj.rearrange("p (h w) -> p h w", h=HT)[
                            :, :, c0 : c0 + cw
                        ],
                        in_=xt[s_, :, :, co_ + c0 : co_ + c0 + cw].rearrange(
                            "h p w -> p h w"
                        ),
                    )

                # gate order: w0 slab + x chunk 0 (the first matmul group's
                # inputs) ahead of w1 in the DMA queue
                w0sj = wp.tile([128, H], bf16, tag="w0", name=None)
                nc.sync.dma_start(out=w0sj, in_=w0t[s_, 0])
                load_x_chunk(0)
                w1sj = wp.tile([128, H], bf16, tag="w1", name=None)
                nc.sync.dma_start(out=w1sj, in_=w1t[s_, 0])
                for ci in range(1, len(ch)):
                    load_x_chunk(ci)

                def xslj(h, c0, cw, _xs=xsj, _W=W_):
                    return _xs[:, h * _W + c0 : h * _W + c0 + cw]

                gates[j] = (xslj, w0sj, w1sj)

            emit_gate(0)
            for jn, (s, co, W) in enumerate(jobs):
                    chunks = _chunk_plan(W)
                    xsl, w0s_first, w1s_first = gates.pop(jn)

                    acts = [
                        apool.tile([128, W], bf16, tag=f"a{i}", name=f"a{i}_{jn}")
                        for i in range(IT)
                    ]

                    def load_w01_slab(which, src, i):
                        slab = wp.tile([128, H], bf16, tag=which, name=None)
                        nc.sync.dma_start(out=slab, in_=src[s, i])
                        return slab

                    # all 16 w2 slabs prefetch with staggered emission: h0-5
                    # during the last Phase A i-section, h6-15 after Phase A
                    w2_tiles = {}

                    def load_w2(h):
                        w2_tiles[h] = wp.tile(
                            [128, I], bf16, tag="w2", bufs=16, name=f"w2_{jn}_{h}"
                        )
                        nc.sync.dma_start(out=w2_tiles[h], in_=w2t[s, h])

                    # Phase A: gate/up projections + silu*up, per i-tile.
                    for i in range(IT):
                        if i == 0:
                            w0s, w1s = w0s_first, w1s_first
                        else:
                            w0s = load_w01_slab("w0", w0t, i)
                            w1s = load_w01_slab("w1", w1t, i)
                        if i == IT - 1:
                            for h in range(6):
                                load_w2(h)
                        for c0, cw in chunks:
                            g = ps.tile([128, 512], f32, tag="g")
                            for h in range(HT):
                                nc.tensor.matmul(
                                    g[:, :cw],
                                    w0s[:, ts(h, 128)],
                                    xsl(h, c0, cw),
                                    start=(h == 0),
                                    stop=(h == HT - 1),
                                )
                            u = ps.tile([128, 512], f32, tag="u")
                            for h in range(HT):
                                nc.tensor.matmul(
                                    u[:, :cw],
                                    w1s[:, ts(h, 128)],
                                    xsl(h, c0, cw),
                                    start=(h == 0),
                                    stop=(h == HT - 1),
                                )
                            a_sl = acts[i][:, c0 : c0 + cw]
                            nc.scalar.activation(a_sl, g[:, :cw], act_fn)
                            nc.vector.tensor_mul(a_sl, a_sl, u[:, :cw])
                            if jn == 0 and i < 2:
                                # filler matmuls execute during the early
                                # DMA-paced trickle so the HAM clock gate
                                # never re-throttles to 1.2 GHz
                                warm_burst(6)

                    for h in range(6, HT):
                        load_w2(h)
                    if jn + 1 < len(jobs):
                        emit_gate(jn + 1)

                    # Phase B: down projection, per h-tile.
                    for h in range(HT):
                        w2s = w2_tiles.pop(h)
                        yc = yp.tile([128, W], bf16, tag="y", bufs=4)
                        for c0, cw in chunks:
                            o = ps.tile([128, 512], f32, tag="o", bufs=4)
                            for i in range(IT):
                                nc.tensor.matmul(
                                    o[:, :cw],
                                    w2s[:, ts(i, 128)],
                                    acts[i][:, c0 : c0 + cw],
                                    start=(i == 0),
                                    stop=(i == IT - 1),
                                )
                            nc.vector.tensor_copy(yc[:, c0 : c0 + cw], o[:, :cw])
                            if jn == len(jobs) - 1 and h == HT - 1:
                                # final h-tile: per-chunk writes shrink the tail
                                nc.scalar.dma_start(
                                    out=yt[s, h, :, co + c0 : co + c0 + cw],
                                    in_=yc[:, c0 : c0 + cw],
                                )
                        # batched y write per h-tile on the Scalar DGE queue
                        if not (jn == len(jobs) - 1 and h == HT - 1):
                            nc.scalar.dma_start(
                                out=yt[s, h, :, co : co + W], in_=yc[:, :W]
                            )
    nc.finalize()
    return nc


def _get_built(jobs, CT):
    key = (tuple(jobs), CT)
    if key not in _BUILT:
        _BUILT[key] = _build(tuple(jobs), CT)
    return _BUILT[key]


def _dequant(w, s):
    """w: [E, O, Iin], s: [E, O, Iin//128] -> dequantized [E, O, Iin]."""
    e, o, iin = w.shape
    return (w.reshape(e, o, iin // BS, BS) * s[..., None]).reshape(e, o, iin)


def _bf16(a):
    import ml_dtypes

    return np.ascontiguousarray(a.astype(ml_dtypes.bfloat16))


def _align(v, m=8):
    return -(-int(v) // m) * m


def _bipartition(widths, cA, cB):
    """Return (SA, SB) slot-index lists with sum(widths[SA]) >= cA and
    sum(widths[SB]) >= cB, or None."""
    n = len(widths)
    best = None
    for mask in range(1, 1 << n):
        sa = [i for i in range(n) if mask >> i & 1]
        sb = [i for i in range(n) if not mask >> i & 1]
        wa = sum(widths[i] for i in sa)
        wb = sum(widths[i] for i in sb)
        if wa >= cA and wb >= cB:
            waste = (wa - cA) + (wb - cB)
            if best is None or waste < best[0]:
                best = (waste, sa, sb)
    return None if best is None else (best[1], best[2])


def _feasible(W, total, pairs, masks):
    ss = masks @ np.array(W)
    for a, b in pairs:
        if not ((ss >= a) & (total - ss >= b)).any():
            return False
    return True


def _find_slot_widths6(pairs):
    """Search for 6 slot widths in [320, 512] (multiples of 4, descending)
    with minimal total such that every core's expert pair fits some slot
    bipartition.  All-<=512 slots mean every job is a single PSUM-bank chunk,
    which minimizes the matmul instruction count."""
    lb = _align(max(a + b for a, b in pairs), 4)
    masks = np.array([[m >> i & 1 for i in range(6)] for m in range(64)])
    lo, hi = 80, 128
    for total in range(lb, lb + 65, 4):
        t4 = total // 4
        for w1 in range(hi, lo - 1, -1):
            for w2 in range(min(w1, t4), lo - 1, -1):
                for w3 in range(min(w2, t4), lo - 1, -1):
                    rem = t4 - w1 - w2 - w3
                    if rem < 3 * lo or rem > 3 * w3:
                        continue
                    for w4 in range(min(w3, rem - 2 * lo), max(lo, -(-rem // 3)) - 1, -1):
                        rem2 = rem - w4
                        for w5 in range(min(w4, rem2 - lo), max(lo, -(-rem2 // 2)) - 1, -1):
                            w6 = rem2 - w5
                            if w6 < lo or w6 > w5:
                                continue
                            W = [w * 4 for w in (w1, w2, w3, w4, w5, w6)]
                            if _feasible(W, total, pairs, masks):
                                return W
    return None


def _greedy_class_match(W, counts_desc):
    """Assign each expert (counts desc) a pair of width classes, min-waste
    first, 8 slots per class.  16 experts x 2 pieces = 32 = 8 cores x 4 slots,
    so a completed match uses every class exactly 8 times."""
    avail = [NCORES] * len(W)
    out = []
    for c in counts_desc:
        best = None
        for i in range(len(W)):
            for j in range(i, len(W)):
                if i == j and avail[i] < 2:
                    continue
                if i != j and (avail[i] < 1 or avail[j] < 1):
                    continue
                sij = W[i] + W[j]
                if sij < c:
                    continue
                if best is None or sij < best[0]:
                    best = (sij, i, j)
        if best is None:
            return None
        _, i, j = best
        avail[i] -= 1
        avail[j] -= 1
        out.append((i, j))
    return out


def _find_global_widths(counts_desc):
    """Search 4 slot widths (multiples of 4, descending) minimizing the
    per-core total, where each expert's columns split across ANY two slots
    on ANY cores (weights are per-slot anyway, so cross-core placement is
    free).  This relaxes the per-core budget from the worst expert-pair sum
    toward the global average."""
    if len(counts_desc) != 2 * NCORES:
        return None
    cmax, cmin = counts_desc[0], counts_desc[-1]
    lo1 = -(-(-(-cmax // 2)) // 4)
    lo = max(32, (cmin // 2) // 4 - 16)
    best = None
    for w1 in range(lo1, lo1 + 10):
        for w2 in range(lo, w1 + 1):
            for w3 in range(lo, w2 + 1):
                for w4 in range(lo, w3 + 1):
                    W = (w1 * 4, w2 * 4, w3 * 4, w4 * 4)
                    if best is not None and sum(W) >= best[0]:
                        continue
                    m = _greedy_class_match(W, counts_desc)
                    if m is not None:
                        best = (sum(W), W, m)
    return best


def _find_slot_widths(pairs):
    """Search for 4 slot widths (multiples of 4, descending) with minimal
    total such that every core's expert pair fits some slot bipartition."""
    lb = _align(max(a + b for a, b in pairs), 4)
    cap1 = _align(max(a for a, _ in pairs), 4) // 4
    masks = np.array([[m >> i & 1 for i in range(4)] for m in range(16)])
    for total in range(lb, lb + 129, 4):
        t4 = total // 4
        cands = []
        for w1 in range(-(-t4 // 4), min(cap1, t4 - 3 * 32) + 1):
            for w2 in range(-(-(t4 - w1) // 3), min(w1, t4 - w1 - 2 * 32) + 1):
                w3lo = -(-(t4 - w1 - w2) // 2)
                w3hi = min(w2, t4 - w1 - w2 - 32)
                for w3 in range(w3lo, w3hi + 1):
                    cands.append((w1, w2, w3, t4 - w1 - w2 - w3))
        if not cands:
            continue
        C = np.array(cands) * 4  # [N, 4]
        ssums = C @ masks.T  # [N, 16]
        ok = np.ones(len(C), dtype=bool)
        for a, b in pairs:
            ok &= ((ssums >= a) & (total - ssums >= b)).any(axis=1)
            if not ok.any():
                break
        if ok.any():
            return [int(v) for v in C[np.argmax(ok)]]
    return None


def kernel(**inputs):
    global LAST_RESULTS
    x = np.ascontiguousarray(np.asarray(inputs["x"], dtype=np.float32))
    sel = np.asarray(inputs["selected_experts"])
    w0 = np.asarray(inputs["w0"], dtype=np.float32)
    s0 = np.asarray(inputs["s0"], dtype=np.float32)
    w1 = np.asarray(inputs["w1"], dtype=np.float32)
    s1 = np.asarray(inputs["s1"], dtype=np.float32)
    w2 = np.asarray(inputs["w2"], dtype=np.float32)
    s2 = np.asarray(inputs["s2"], dtype=np.float32)

    t, k = sel.shape
    assert (t, k) == (T, TOPK) and x.shape == (T, H)

    # ---- host-side dispatch: unique tokens per expert ----
    cols = []
    for e in range(E):
        cols.append(np.nonzero((sel == e).any(axis=1))[0])
    counts = np.array([len(c) for c in cols])

    # Pair largest with smallest expert per core (balances pair sums).
    order = np.argsort(-counts, kind="stable")
    pair_of = [(int(order[c]), int(order[2 * NCORES - 1 - c])) for c in range(NCORES)]
    pairs = [(int(counts[a]), int(counts[b])) for a, b in pair_of]
    counts_desc = [int(counts[e]) for e in order]

    gm = _find_global_widths(counts_desc) if counts.max() <= MAX_W else None
    if gm is not None:
        # cross-core expert splitting: each expert's columns go to one slot
        # of class i on some core and one of class j on another
        _, Wdesc, match = gm
        # smallest slot first: its chunk 0 gates the very first matmul group
        class_to_slot = {3: 0, 0: 1, 1: 2, 2: 3}
        widths = [Wdesc[3], Wdesc[0], Wdesc[1], Wdesc[2]]
        CT = max(widths)
        S = 4
        jobs = tuple((s, 0, w) for s, w in enumerate(widths))
        slot_expert = [[None] * S for _ in range(NCORES)]
        slot_tokens = [[None] * S for _ in range(NCORES)]
        src_core = np.zeros((E, T), dtype=np.int64)
        src_slot = np.zeros((E, T), dtype=np.int64)
        src_col = np.zeros((E, T), dtype=np.int64)
        next_core = [0] * 4
        for idx, e in enumerate(order):
            e = int(e)
            i, j = match[idx]
            a = min(Wdesc[i], counts_desc[idx])
            toks = cols[e]
            for cls, seg in ((i, toks[:a]), (j, toks[a:])):
                core = next_core[cls]
                next_core[cls] += 1
                s = class_to_slot[cls]
                slot_expert[core][s] = e
                slot_tokens[core][s] = seg
                src_core[e, seg] = core
                src_slot[e, seg] = s
                src_col[e, seg] = np.arange(len(seg))
    else:
        widths = _find_slot_widths(pairs) if counts.max() <= MAX_W else None
        if widths is None:
            # fallback: classic 2-slot layout
            widths = [
                max(256, _align(max(p[0] for p in pairs))),
                max(256, _align(max(p[1] for p in pairs))),
            ]
            if max(widths) > MAX_W:
                cmax = int(counts.max())
                passes = math.ceil(cmax / MAX_W)
                W = max(256, _align(math.ceil(cmax / passes)))
                CT = W * passes
                jobs = tuple((s, cp * W, W) for s in range(2) for cp in range(passes))
            else:
                CT = max(widths)
                jobs = tuple((s, 0, w) for s, w in enumerate(widths))
        else:
            # smallest slot first: its chunk 0 gates the very first matmul group
            widths = sorted(widths)[:1] + sorted(widths)[1:][::-1]
            CT = max(widths)
            jobs = tuple((s, 0, w) for s, w in enumerate(widths))
        S = max(j[0] for j in jobs) + 1

        # Per-core slot assignment: slot s of core c serves slot_expert[c][s]
        # over global tokens slot_tokens[c][s].
        slot_expert = [[None] * S for _ in range(NCORES)]
        slot_tokens = [[None] * S for _ in range(NCORES)]
        src_core = np.zeros((E, T), dtype=np.int64)
        src_slot = np.zeros((E, T), dtype=np.int64)
        src_col = np.zeros((E, T), dtype=np.int64)
        slot_caps = [sum(w for s2_, _, w in jobs if s2_ == s) for s in range(S)]
        for c in range(NCORES):
            eA, eB = pair_of[c]
            bp = _bipartition(slot_caps, int(counts[eA]), int(counts[eB]))
            assert bp is not None
            for ex, slots in ((eA, bp[0]), (eB, bp[1])):
                toks = cols[ex]
                off = 0
                for s in sorted(slots):
                    n = min(slot_caps[s], len(toks) - off)
                    seg = toks[off : off + n]
                    slot_expert[c][s] = ex
                    slot_tokens[c][s] = seg
                    src_core[ex, seg] = c
                    src_slot[ex, seg] = s
                    src_col[ex, seg] = np.arange(n)
                    off += n
                assert off == len(toks)

    # ---- dequantize + slab-major weight prep (host) ----
    W0 = _dequant(w0, s0)  # [E, I, H]
    W1 = _dequant(w1, s1)  # [E, I, H]
    W2 = _dequant(w2, s2)  # [E, H, I]
    w0n = _bf16(W0.reshape(E, IT, 128, HT, 128).transpose(0, 1, 4, 3, 2).reshape(E, IT, 128, H))
    w1n = _bf16(W1.reshape(E, IT, 128, HT, 128).transpose(0, 1, 4, 3, 2).reshape(E, IT, 128, H))
    w2n = _bf16(W2.reshape(E, HT, 128, IT, 128).transpose(0, 1, 4, 3, 2).reshape(E, HT, 128, I))

    in_maps = []
    for c in range(NCORES):
        xt_c = np.zeros((S, H, CT), dtype=np.float32)
        exps = []
        for s in range(S):
            seg = slot_tokens[c][s]
            ex = slot_expert[c][s]
            exps.append(0 if ex is None else ex)
            if seg is not None and len(seg):
                xt_c[s, :, : len(seg)] = x[seg].T
        in_maps.append(
            {
                "xt": _bf16(xt_c.reshape(S, HT, 128, CT)),
                "w0t": np.ascontiguousarray(w0n[exps]),
                "w1t": np.ascontiguousarray(w1n[exps]),
                "w2t": np.ascontiguousarray(w2n[exps]),
            }
        )

    nc = _get_built(jobs, CT)
    from concourse.bass_utils import run_bass_kernel_spmd

    res = run_bass_kernel_spmd(nc, in_maps, list(range(NCORES)))
    LAST_RESULTS = res

    # Yall[core, slot] = [H, CT]
    Yall = np.empty((NCORES, S, H, CT), dtype=np.float32)
    for c in range(NCORES):
        Yall[c] = np.asarray(res.results[c]["yt"]).astype(np.float32).reshape(S, H, CT)

    # ---- scatter back to [T, K, H] ----
    e_flat = sel.reshape(-1).astype(np.int64)
    t_flat = np.repeat(np.arange(T, dtype=np.int64), TOPK)
    out = Yall[
        src_core[e_flat, t_flat], src_slot[e_flat, t_flat], :, src_col[e_flat, t_flat]
    ]  # [T*K, H]
    return np.ascontiguousarray(out.reshape(T, TOPK, H), dtype=np.float32)


# revision 44
# speedup vs baseline: 1.1977x; 1.1977x over previous
"""DeepseekMoE block-quantized MoE kernel for 8 Trainium2 NeuronCores.

Strategy (expert-parallel with host-side dispatch):
  - The routing table (selected_experts) is known on the host before launch,
    so the all-to-all "dispatch" is done on the host: for each expert e we
    gather the unique tokens routed to it (dedup across the top-k slots),
    transpose to [H, n_e], and split across per-core job slots.
  - Each core hosts 2 experts.  The SPMD program runs S jobs of fixed
    widths W[0..S-1]; each job slot is fed one expert's weights and a
    window of its token columns.  With S=4 narrow slots a core can split
    its two experts across a per-core bipartition of the slots, so the
    total padded width per core approaches the max expert-pair sum
    (2656 vs 2704 for the classic 2-slot layout) - all host-side data
    routing, no program branching.
  - Each job runs a dense 3-matmul MLP (gate/up -> silu*up -> down) in
    x^T / act^T layout so no on-device transposes are needed.
  - Block-dequantization (w * repeat(s, 128)) is folded into the host-side
    weight preparation.
  - All tensors are bf16 on device (PSUM accumulation stays fp32): same
    1 col/cycle PE streaming rate as float32r, but half the DMA bytes and
    SBUF footprint, and LDWEIGHTS gets fast-weight-load (2x).
  - Weights are stored in DRAM in slab-major layout ([slab, 128, free])
    so each weight-slab DMA is 128 descriptors of 4KB/2.8KB contiguous
    instead of 2048 descriptors of 512B (the DMA engines are
    descriptor-rate-bound, not byte-bound).
  - x loads are one 3D-AP DMA per chunk (DMA issue costs ~600ns each on
    the issuing engine; packets of one DMA round-robin across all 16
    engines anyway).  y writes are batched per h-tile and issued from the
    Scalar engine's DGE queue so their CAST-waits never head-of-line
    block the input loads on the Sync queue.
  - ~100 warmup matmuls on scratch tiles run while the first inputs
    stream in, so the PE's HAM clock gate reaches 2.4 GHz before real
    work starts.
  - The host scatters the per-slot outputs back to [T, K, H].
"""

import math

import numpy as np

T = 4096
TOPK = 6
E = 16
H = 2048
I = 1408
BS = 128           # quant block size
HT = H // 128      # 16 h-tiles
IT = I // 128      # 11 i-tiles
NCORES = 8
# SBUF bound per partition: 62*W bytes of W-dependent tiles (x, acts, yc)
# plus ~60KB of weight staging must stay under ~200KB.
MAX_W = 2240

_BUILT = {}
LAST_RESULTS = None  # stashed BassKernelResults for external harnesses


def _chunk_plan(width):
    """Split `width` columns into PSUM-bank-sized chunks (<=512), each >=256
    when width allows (keeps every matmul well above the LDWEIGHTS shadow)."""
    if width <= 512:
        return [(0, width)]
    n = -(-width // 512)
    base = (width // n) // 8 * 8
    rem8 = (width - n * base) // 8
    out, off = [], 0
    for j in range(n):
        w = base + (8 if j < rem8 else 0)
        if j == n - 1:
            w = width - off
        out.append((off, w))
        off += w
    return out


def _build(jobs, CT):
    """Build the SPMD Bass program.  `jobs` is a tuple of
    (slot, col_offset, width): each job runs one expert slot's MLP over a
    window of `width` token columns; CT is the column capacity of xt/yt."""
    import concourse.bacc as bacc
    import concourse.mybir as mybir
    from concourse.bass import ts
    from concourse.tile import TileContext

    f32 = mybir.dt.float32
    bf16 = mybir.dt.bfloat16
    AF = mybir.ActivationFunctionType
    import os as _os

    act_fn = (
        AF.Sigmoid if _os.environ.get("KERNEL_SIM_SIGMOID") else AF.Silu
    )  # CoreSim lacks Silu; HW path always uses Silu

    S = max(j[0] for j in jobs) + 1
    nc = bacc.Bacc()
    xt = nc.declare_dram_parameter("xt", [S, HT, 128, CT], bf16, isOutput=False)
    # slab-major weights: w0t/w1t[s, it, p, hb*128+j] = Wdq[it*128+j, hb*128+p]
    w0t = nc.declare_dram_parameter("w0t", [S, IT, 128, H], bf16, isOutput=False)
    w1t = nc.declare_dram_parameter("w1t", [S, IT, 128, H], bf16, isOutput=False)
    # w2t[s, ht, p, it*128+j] = W2dq[ht*128+j, it*128+p]
    w2t = nc.declare_dram_parameter("w2t", [S, HT, 128, I], bf16, isOutput=False)
    yt = nc.declare_dram_parameter("yt", [S, HT, 128, CT], bf16, isOutput=True)

    with TileContext(nc) as tc:
        with (
            tc.tile_pool(name="xp", bufs=1) as xp,
            tc.tile_pool(name="ap", bufs=1) as apool,
            tc.tile_pool(name="wp", bufs=2) as wp,
            tc.tile_pool(name="yp", bufs=3) as yp,
            tc.tile_pool(name="ps", bufs=2, space="PSUM") as ps,
        ):
            # PE warmup: dummy matmuls on scratch tiles while the first x/w
            # DMAs stream in, so the HAM clock gate reaches 2.4 GHz before
            # real matmuls start (otherwise the first ~3.4us run at 1.2 GHz).
            warm_sb = xp.tile([128, 192], bf16, tag="warm")
            nc.vector.memset(warm_sb, 0.0)
            warm_ps = ps.tile([128, 512], f32, tag="o", bufs=4)
            for _ in range(22):
                nc.tensor.matmul(
                    warm_ps[:, :192], warm_sb[:, :128], warm_sb,
                    start=True, stop=True,
                )

            # "gate" loads for job j: w0/w1 slab 0 + all x chunks.  Emitted
            # during job j-1's Phase B (after its w2 slabs) so the transfers
            # complete before job j's first matmul group needs them.
            gates = {}

            def emit_gate(j):
                s_, co_, W_ = jobs[j]
                ch = _chunk_plan(W_)
                if j == 0:
                    # job 0's x in four independent tiles (4 h-tiles each):
                    # Tile tracks dependencies per tile, so the first matmul
                    # group starts after w0 + one 315KB part instead of the
                    # whole chunk (real work from ~11us instead of ~18us)
                    parts = [
                        xp.tile([128, 4 * W_], bf16, tag=f"xg{pp}", name=f"xg{pp}_0")
                        for pp in range(4)
                    ]

                    def load_part_chunk(pp, ci):
                        c0, cw = ch[ci]
                        nc.sync.dma_start(
                            out=parts[pp].rearrange("p (h w) -> p h w", h=4)[
                                :, :, c0 : c0 + cw
                            ],
                            in_=xt[
                                s_, 4 * pp : 4 * pp + 4, :, co_ + c0 : co_ + c0 + cw
                            ].rearrange("h p w -> p h w"),
                        )

                    w0sj = wp.tile([128, H], bf16, tag="w0", name=None)
                    nc.sync.dma_start(out=w0sj, in_=w0t[s_, 0])
                    for pp in range(4):
                        load_part_chunk(pp, 0)
                    w1sj = wp.tile([128, H], bf16, tag="w1", name=None)
                    nc.sync.dma_start(out=w1sj, in_=w1t[s_, 0])
                    for ci in range(1, len(ch)):
                        for pp in range(4):
                            load_part_chunk(pp, ci)

                    def xslj(h, c0, cw, _p=parts, _W=W_):
                        return _p[h // 4][
                            :, (h % 4) * _W + c0 : (h % 4) * _W + c0 + cw
                        ]

                    gates[j] = (xslj, w0sj, w1sj)
                    return
                xsj = xp.tile([128, HT * W_], bf16, tag="x", name=f"x_{j}")

                def load_x_chunk(ci):
                    c0, cw = ch[ci]
                    nc.sync.dma_start(
                        out=xsj.rearrange("p (h w) -> p h w", h=HT)[
                            :, :, c0 : c0 + cw
                        ],
                        in_=xt[s_, :, :, co_ + c0 : co_ + c0 + cw].rearrange(
                            "h p w -> p h w"
                        ),
                    )

                # gate order: w0 slab + x chunk 0 (the first matmul group's
                # inputs) ahead of w1 in the DMA queue
                w0sj = wp.tile([128, H], bf16, tag="w0", name=None)
                nc.sync.dma_start(out=w0sj, in_=w0t[s_, 0])
                load_x_chunk(0)
                w1sj = wp.tile([128, H], bf16, tag="w1", name=None)
                nc.sync.dma_start(out=w1sj, in_=w1t[s_, 0])
                for ci in range(1, len(ch)):
                    load_x_chunk(ci)

                def xslj(h, c0, cw, _xs=xsj, _W=W_):
                    return _xs[:, h * _W + c0 : h * _W + c0 + cw]

                gates[j] = (xslj, w0sj, w1sj)

            emit_gate(0)
            for jn, (s, co, W) in enumerate(jobs):
                    chunks = _chunk_plan(W)
                    xsl, w0s_first, w1s_first = gates.pop(jn)

                    acts = [
                        apool.tile([128, W], bf16, tag=f"a{i}", name=f"a{i}_{jn}")
                        for i in range(IT)
                    ]

                    def load_w01_slab(which, src, i):
                        slab = wp.tile([128, H], bf16, tag=which, name=None)
                        nc.sync.dma_start(out=slab, in_=src[s, i])
                        return slab

                    # all 16 w2 slabs prefetch with staggered emission: h0-5
                    # during the last Phase A i-section, h6-15 after Phase A
                    w2_tiles = {}

                    def load_w2(h):
                        w2_tiles[h] = wp.tile(
                            [128, I], bf16, tag="w2", bufs=16, name=f"w2_{jn}_{h}"
                        )
                        nc.sync.dma_start(out=w2_tiles[h], in_=w2t[s, h])

                    # Phase A: gate/up projections + silu*up, per i-tile.
                    for i in range(IT):
                        if i == 0:
                            w0s, w1s = w0s_first, w1s_first
                        else:
                            w0s = load_w01_slab("w0", w0t, i)
                            w1s = load_w01_slab("w1", w1t, i)
                        if i == IT - 1:
                            for h in range(6):
                                load_w2(h)
                        for c0, cw in chunks:
                            g = ps.tile([128, 512], f32, tag="g")
                            for h in range(HT):
                                nc.tensor.matmul(
                                    g[:, :cw],
                                    w0s[:, ts(h, 128)],
                                    xsl(h, c0, cw),
                                    start=(h == 0),
                                    stop=(h == HT - 1),
                                )
                            u = ps.tile([128, 512], f32, tag="u")
                            for h in range(HT):
                                nc.tensor.matmul(
                                    u[:, :cw],
                                    w1s[:, ts(h, 128)],
                                    xsl(h, c0, cw),
                                    start=(h == 0),
                                    stop=(h == HT - 1),
                                )
                            a_sl = acts[i][:, c0 : c0 + cw]
                            nc.scalar.activation(a_sl, g[:, :cw], act_fn)
                            nc.vector.tensor_mul(a_sl, a_sl, u[:, :cw])

                    for h in range(6, HT):
                        load_w2(h)
                    if jn + 1 < len(jobs):
                        emit_gate(jn + 1)

                    # Phase B: down projection, per h-tile.
                    for h in range(HT):
                        w2s = w2_tiles.pop(h)
                        yc = yp.tile([128, W], bf16, tag="y", bufs=4)
                        for c0, cw in chunks:
                            o = ps.tile([128, 512], f32, tag="o", bufs=4)
                            for i in range(IT):
                                nc.tensor.matmul(
                                    o[:, :cw],
                                    w2s[:, ts(i, 128)],
                                    acts[i][:, c0 : c0 + cw],
                                    start=(i == 0),
                                    stop=(i == IT - 1),
                                )
                            nc.vector.tensor_copy(yc[:, c0 : c0 + cw], o[:, :cw])
                            if jn == len(jobs) - 1 and h == HT - 1:
                                # final h-tile: per-chunk writes shrink the tail
                                nc.scalar.dma_start(
                                    out=yt[s, h, :, co + c0 : co + c0 + cw],
                                    in_=yc[:, c0 : c0 + cw],
                                )
                        # batched y write per h-tile on the Scalar DGE queue
                        if not (jn == len(jobs) - 1 and h == HT - 1):
                            nc.scalar.dma_start(
                                out=yt[s, h, :, co : co + W], in_=yc[:, :W]
                            )
    nc.finalize()
    return nc


def _get_built(jobs, CT):
    key = (tuple(jobs), CT)
    if key not in _BUILT:
        _BUILT[key] = _build(tuple(jobs), CT)
    return _BUILT[key]


def _dequant(w, s):
    """w: [E, O, Iin], s: [E, O, Iin//128] -> dequantized [E, O, Iin]."""
    e, o, iin = w.shape
    return (w.reshape(e, o, iin // BS, BS) * s[..., None]).reshape(e, o, iin)


def _bf16(a):
    import ml_dtypes

    return np.ascontiguousarray(a.astype(ml_dtypes.bfloat16))


def _align(v, m=8):
    return -(-int(v) // m) * m


def _bipartition(widths, cA, cB):
    """Return (SA, SB) slot-index lists with sum(widths[SA]) >= cA and
    sum(widths[SB]) >= cB, or None."""
    n = len(widths)
    best = None
    for mask in range(1, 1 << n):
        sa = [i for i in range(n) if mask >> i & 1]
        sb = [i for i in range(n) if not mask >> i & 1]
        wa = sum(widths[i] for i in sa)
        wb = sum(widths[i] for i in sb)
        if wa >= cA and wb >= cB:
            waste = (wa - cA) + (wb - cB)
            if best is None or waste < best[0]:
                best = (waste, sa, sb)
    return None if best is None else (best[1], best[2])


def _feasible(W, total, pairs, masks):
    ss = masks @ np.array(W)
    for a, b in pairs:
        if not ((ss >= a) & (total - ss >= b)).any():
            return False
    return True


def _find_slot_widths6(pairs):
    """Search for 6 slot widths in [320, 512] (multiples of 4, descending)
    with minimal total such that every core's expert pair fits some slot
    bipartition.  All-<=512 slots mean every job is a single PSUM-bank chunk,
    which minimizes the matmul instruction count."""
    lb = _align(max(a + b for a, b in pairs), 4)
    masks = np.array([[m >> i & 1 for i in range(6)] for m in range(64)])
    lo, hi = 80, 128
    for total in range(lb, lb + 65, 4):
        t4 = total // 4
        for w1 in range(hi, lo - 1, -1):
            for w2 in range(min(w1, t4), lo - 1, -1):
                for w3 in range(min(w2, t4), lo - 1, -1):
                    rem = t4 - w1 - w2 - w3
                    if rem < 3 * lo or rem > 3 * w3:
                        continue
                    for w4 in range(min(w3, rem - 2 * lo), max(lo, -(-rem // 3)) - 1, -1):
                        rem2 = rem - w4
                        for w5 in range(min(w4, rem2 - lo), max(lo, -(-rem2 // 2)) - 1, -1):
                            w6 = rem2 - w5
                            if w6 < lo or w6 > w5:
                                continue
                            W = [w * 4 for w in (w1, w2, w3, w4, w5, w6)]
                            if _feasible(W, total, pairs, masks):
                                return W
    return None


def _greedy_class_match(W, counts_desc):
    """Assign each expert (counts desc) a pair of width classes, min-waste
    first, 8 slots per class.  16 experts x 2 pieces = 32 = 8 cores x 4 slots,
    so a completed match uses every class exactly 8 times."""
    avail = [NCORES] * len(W)
    out = []
    for c in counts_desc:
        best = None
        for i in range(len(W)):
            for j in range(i, len(W)):
                if i == j and avail[i] < 2:
                    continue
                if i != j and (avail[i] < 1 or avail[j] < 1):
                    continue
                sij = W[i] + W[j]
                if sij < c:
                    continue
                if best is None or sij < best[0]:
                    best = (sij, i, j)
        if best is None:
            return None
        _, i, j = best
        avail[i] -= 1
        avail[j] -= 1
        out.append((i, j))
    return out


def _find_global_widths(counts_desc):
    """Search 4 slot widths (multiples of 4, descending) minimizing the
    per-core total, where each expert's columns split across ANY two slots
    on ANY cores (weights are per-slot anyway, so cross-core placement is
    free).  This relaxes the per-core budget from the worst expert-pair sum
    toward the global average."""
    if len(counts_desc) != 2 * NCORES:
        return None
    cmax, cmin = counts_desc[0], counts_desc[-1]
    lo1 = -(-(-(-cmax // 2)) // 4)
    lo = max(32, (cmin // 2) // 4 - 16)
    best = None
    for w1 in range(lo1, lo1 + 10):
        for w2 in range(lo, w1 + 1):
            for w3 in range(lo, w2 + 1):
                for w4 in range(lo, w3 + 1):
                    W = (w1 * 4, w2 * 4, w3 * 4, w4 * 4)
                    if best is not None and sum(W) >= best[0]:
                        continue
                    m = _greedy_class_match(W, counts_desc)
                    if m is not None:
                        best = (sum(W), W, m)
    return best


def _find_slot_widths(pairs):
    """Search for 4 slot widths (multiples of 4, descending) with minimal
    total such that every core's expert pair fits some slot bipartition."""
    lb = _align(max(a + b for a, b in pairs), 4)
    cap1 = _align(max(a for a, _ in pairs), 4) // 4
    masks = np.array([[m >> i & 1 for i in range(4)] for m in range(16)])
    for total in range(lb, lb + 129, 4):
        t4 = total // 4
        cands = []
        for w1 in range(-(-t4 // 4), min(cap1, t4 - 3 * 32) + 1):
            for w2 in range(-(-(t4 - w1) // 3), min(w1, t4 - w1 - 2 * 32) + 1):
                w3lo = -(-(t4 - w1 - w2) // 2)
                w3hi = min(w2, t4 - w1 - w2 - 32)
                for w3 in range(w3lo, w3hi + 1):
                    cands.append((w1, w2, w3, t4 - w1 - w2 - w3))
        if not cands:
            continue
        C = np.array(cands) * 4  # [N, 4]
        ssums = C @ masks.T  # [N, 16]
        ok = np.ones(len(C), dtype=bool)
        for a, b in pairs:
            ok &= ((ssums >= a) & (total - ssums >= b)).any(axis=1)
            if not ok.any():
                break
        if ok.any():
            return [int(v) for v in C[np.argmax(ok)]]
    return None


def kernel(**inputs):
    global LAST_RESULTS
    x = np.ascontiguousarray(np.asarray(inputs["x"], dtype=np.float32))
    sel = np.asarray(inputs["selected_experts"])
    w0 = np.asarray(inputs["w0"], dtype=np.float32)
    s0 = np.asarray(inputs["s0"], dtype=np.float32)
    w1 = np.asarray(inputs["w1"], dtype=np.float32)
    s1 = np.asarray(inputs["s1"], dtype=np.float32)
    w2 = np.asarray(inputs["w2"], dtype=np.float32)
    s2 = np.asarray(inputs["s2"], dtype=np.float32)

    t, k = sel.shape
    assert (t, k) == (T, TOPK) and x.shape == (T, H)

    # ---- host-side dispatch: unique tokens per expert ----
    cols = []
    for e in range(E):
        cols.append(np.nonzero((sel == e).any(axis=1))[0])
    counts = np.array([len(c) for c in cols])

    # Pair largest with smallest expert per core (balances pair sums).
    order = np.argsort(-counts, kind="stable")
    pair_of = [(int(order[c]), int(order[2 * NCORES - 1 - c])) for c in range(NCORES)]
    pairs = [(int(counts[a]), int(counts[b])) for a, b in pair_of]
    counts_desc = [int(counts[e]) for e in order]

    gm = _find_global_widths(counts_desc) if counts.max() <= MAX_W else None
    if gm is not None:
        # cross-core expert splitting: each expert's columns go to one slot
        # of class i on some core and one of class j on another
        _, Wdesc, match = gm
        # smallest slot first: its chunk 0 gates the very first matmul group
        class_to_slot = {3: 0, 0: 1, 1: 2, 2: 3}
        widths = [Wdesc[3], Wdesc[0], Wdesc[1], Wdesc[2]]
        CT = max(widths)
        S = 4
        jobs = tuple((s, 0, w) for s, w in enumerate(widths))
        slot_expert = [[None] * S for _ in range(NCORES)]
        slot_tokens = [[None] * S for _ in range(NCORES)]
        src_core = np.zeros((E, T), dtype=np.int64)
        src_slot = np.zeros((E, T), dtype=np.int64)
        src_col = np.zeros((E, T), dtype=np.int64)
        next_core = [0] * 4
        for idx, e in enumerate(order):
            e = int(e)
            i, j = match[idx]
            a = min(Wdesc[i], counts_desc[idx])
            toks = cols[e]
            for cls, seg in ((i, toks[:a]), (j, toks[a:])):
                core = next_core[cls]
                next_core[cls] += 1
                s = class_to_slot[cls]
                slot_expert[core][s] = e
                slot_tokens[core][s] = seg
                src_core[e, seg] = core
                src_slot[e, seg] = s
                src_col[e, seg] = np.arange(len(seg))
    else:
        widths = _find_slot_widths(pairs) if counts.max() <= MAX_W else None
        if widths is None:
            # fallback: classic 2-slot layout
            widths = [
                max(256, _align(max(p[0] for p in pairs))),
                max(256, _align(max(p[1] for p in pairs))),
            ]
            if max(widths) > MAX_W:
                cmax = int(counts.max())
                passes = math.ceil(cmax / MAX_W)
                W = max(256, _align(math.ceil(cmax / passes)))
                CT = W * passes
                jobs = tuple((s, cp * W, W) for s in range(2) for cp in range(passes))
            else:
                CT = max(widths)
                jobs = tuple((s, 0, w) for s, w in enumerate(widths))
        else:
            # smallest slot first: its chunk 0 gates the very first matmul group
            widths = sorted(widths)[:1] + sorted(widths)[1:][::-1]
            CT = max(widths)
            jobs = tuple((s, 0, w) for s, w in enumerate(widths))
        S = max(j[0] for j in jobs) + 1

        # Per-core slot assignment: slot s of core c serves slot_expert[c][s]
        # over global tokens slot_tokens[c][s].
        slot_expert = [[None] * S for _ in range(NCORES)]
        slot_tokens = [[None] * S for _ in range(NCORES)]
        src_core = np.zeros((E, T), dtype=np.int64)
        src_slot = np.zeros((E, T), dtype=np.int64)
        src_col = np.zeros((E, T), dtype=np.int64)
        slot_caps = [sum(w for s2_, _, w in jobs if s2_ == s) for s in range(S)]
        for c in range(NCORES):
            eA, eB = pair_of[c]
            bp = _bipartition(slot_caps, int(counts[eA]), int(counts[eB]))
            assert bp is not None
            for ex, slots in ((eA, bp[0]), (eB, bp[1])):
                toks = cols[ex]
                off = 0
                for s in sorted(slots):
                    n = min(slot_caps[s], len(toks) - off)
                    seg = toks[off : off + n]
                    slot_expert[c][s] = ex
                    slot_tokens[c][s] = seg
                    src_core[ex, seg] = c
                    src_slot[ex, seg] = s
                    src_col[ex, seg] = np.arange(n)
                    off += n
                assert off == len(toks)

    # ---- dequantize + slab-major weight prep (host) ----
    W0 = _dequant(w0, s0)  # [E, I, H]
    W1 = _dequant(w1, s1)  # [E, I, H]
    W2 = _dequant(w2, s2)  # [E, H, I]
    w0n = _bf16(W0.reshape(E, IT, 128, HT, 128).transpose(0, 1, 4, 3, 2).reshape(E, IT, 128, H))
    w1n = _bf16(W1.reshape(E, IT, 128, HT, 128).transpose(0, 1, 4, 3, 2).reshape(E, IT, 128, H))
    w2n = _bf16(W2.reshape(E, HT, 128, IT, 128).transpose(0, 1, 4, 3, 2).reshape(E, HT, 128, I))

    in_maps = []
    for c in range(NCORES):
        xt_c = np.zeros((S, H, CT), dtype=np.float32)
        exps = []
        for s in range(S):
            seg = slot_tokens[c][s]
            ex = slot_expert[c][s]
            exps.append(0 if ex is None else ex)
            if seg is not None and len(seg):
                xt_c[s, :, : len(seg)] = x[seg].T
        in_maps.append(
            {
                "xt": _bf16(xt_c.reshape(S, HT, 128, CT)),
                "w0t": np.ascontiguousarray(w0n[exps]),
                "w1t": np.ascontiguousarray(w1n[exps]),
                "w2t": np.ascontiguousarray(w2n[exps]),
            }
        )

    nc = _get_built(jobs, CT)
    from concourse.bass_utils import run_bass_kernel_spmd

    res = run_bass_kernel_spmd(nc, in_maps, list(range(NCORES)))
    LAST_RESULTS = res

    # Yall[core, slot] = [H, CT]
    Yall = np.empty((NCORES, S, H, CT), dtype=np.float32)
    for c in range(NCORES):
        Yall[c] = np.asarray(res.results[c]["yt"]).astype(np.float32).reshape(S, H, CT)

    # ---- scatter back to [T, K, H] ----
    e_flat = sel.reshape(-1).astype(np.int64)
    t_flat = np.repeat(np.arange(T, dtype=np.int64), TOPK)
    out = Yall[
        src_core[e_flat, t_flat], src_slot[e_flat, t_flat], :, src_col[e_flat, t_flat]
    ]  # [T*K, H]
    return np.ascontiguousarray(out.reshape(T, TOPK, H), dtype=np.float32)
